# revision 29
# baseline (speedup 1.0000x reference)
"""Trainium2 Bass kernel: syllable-pattern logits mask (haiku-chain).

Computes, for full inputs
    logits           [32, 64, 32000] f32
    decoder_input    [32, 64]        i32
    word2syllables   [32000]         i32
    sample_n_to_check scalar (== 1)
the reference:
    rs   = remain_syllables(decoder_input, word2syllables)     # [S, B]
    out  = where(word2syllables[v] > rs[s, b], -inf, logits)

Strategy: data-parallel over the batch dim across 8 NeuronCores (8 batch
columns per core).  Per core the kernel is a DMA-roofline streaming
masked-fill (read 32.75 MB + write 32.75 MB):

  * The per-(s,b) syllable recurrence is computed on-device in closed
    form (no 32-step serial loop) with tensor_tensor_scan cumsum /
    running-max over the sequence dim, producing a per-row threshold
    column q4[p] = 4*rs + 2 for the 256 (s,b) rows.
  * word2syllables is broadcast across all 128 partitions once via a
    K=1 TensorE matmul against a constant [1,128] vector of 4.0s
    (PSUM), then cast to a persistent bf16 [128, 32000] SBUF tile
    holding 4*w2s (exact in bf16).
  * Main loop per [128, 4000] tile:
        h  = (4*w2s[v] - q4[p]) * -1.9e38      # one dual-op tensor_scalar
        out = min(logits, h)                   # one tensor_tensor
    |4*w2s - q4| >= 2, so the multiply overflows to exactly +/-inf:
    +inf where kept (min passes logits through), -inf where masked.
"""

import numpy as np

S, B, V = 32, 64, 32000
NCORES = 8
BC = B // NCORES          # batch columns per core
ROWS = S * BC             # (s, b) rows per core
SEP = 2
# Streaming chunk plan: a few small warm-up chunks let the tiny-phase
# DMA hops interleave with the logits stream before the big chunks
# monopolize the DMA engines.
CHUNKS = [4000] * 8
assert sum(CHUNKS) == V
BIGNEG = -1.9e38          # finite f32; *|d|>=2 overflows to +/-inf
HOLD_K = 4                # in-DMA index held until thresholds land

_CACHE = {}


def _build_nc():
    import concourse.bass as bass
    import concourse.bacc as bacc
    import concourse.mybir as mybir
    from concourse.tile import TileContext

    f32 = mybir.dt.float32
    i32 = mybir.dt.int32
    bf16 = mybir.dt.bfloat16
    Alu = mybir.AluOpType

    nc = bacc.Bacc("TRN2", debug=False)
    lg = nc.dram_tensor("logits", [ROWS, V], f32, kind="ExternalInput")
    dec = nc.dram_tensor("dec", [S, BC], i32, kind="ExternalInput")
    w2s = nc.dram_tensor("w2s", [V], i32, kind="ExternalInput")
    out = nc.dram_tensor("out", [ROWS, V], f32, kind="ExternalOutput")

    with TileContext(nc) as tc:
        with (
            tc.tile_pool(name="lt_pool", bufs=6) as lt_pool,
            tc.tile_pool(name="h_pool", bufs=4) as h_pool,
            tc.tile_pool(name="w_pool", bufs=1) as w_pool,
            tc.tile_pool(name="tiny", bufs=1) as tiny,
            tc.tile_pool(name="psum", bufs=2, space="PSUM") as psum_pool,
            tc.tile_pool(name="dram", bufs=1, space="DRAM") as dram_pool,
        ):
            # Start streaming immediately: the first two logits tiles have
            # no dependencies, so their in-DMAs lead the gpsimd queue.
            lgf = lg.ap()
            outf = out.ap()
            col0 = []
            acc = 0
            for w in CHUNKS:
                col0.append(acc)
                acc += w
            tile_order = [(ci, sb) for ci in range(len(CHUNKS))
                          for sb in range(2)]
            lts = {}
            for k in (0, 1):
                ci, sb = tile_order[k]
                c0, w = col0[ci], CHUNKS[ci]
                lt = lt_pool.tile([128, w], f32, tag=f"lt{w}",
                                  name=f"lt{k}", bufs=4 if w < 4000 else 6)
                nc.gpsimd.dma_start(
                    out=lt, in_=lgf[sb * 128:(sb + 1) * 128, c0:c0 + w],
                )
                lts[k] = lt

            # ---- tiny phase: closed-form recurrence in [BC, S] layout.
            # Critical path: dec_bs load -> per-element gather of
            # w2s[dec] -> short DVE chain -> DRAM bounce into the [128, 2]
            # per-row threshold columns. Ops that depend only on dec_bs are
            # emitted first so DVE starts before the gather lands.
            dec_bs = tiny.tile([BC, S], i32)
            nc.sync.dma_start(out=dec_bs, in_=dec.ap().rearrange("s b -> b s"))
            # HW indirect DMA gathers one ROW per partition (one index per
            # partition; CoreSim's per-element offsets do NOT match HW), so
            # gather w2s[dec] as two [128,1] column gathers in row-major
            # (s*BC+b) order, then bounce through DRAM into [BC, S].
            dec_flat = dec.ap().rearrange("s b -> (s b)")
            w2s_2d = w2s.ap().rearrange("(v one) -> v one", one=1)
            c_scr = dram_pool.tile([ROWS], i32)
            for ti in range(2):
                it = tiny.tile([128, 1], i32, tag=f"it{ti}")
                nc.sync.dma_start(out=it, in_=dec_flat[ti * 128:(ti + 1) * 128])
                cc = tiny.tile([128, 1], i32, tag=f"cc{ti}")
                nc.gpsimd.indirect_dma_start(
                    out=cc,
                    out_offset=None,
                    in_=w2s_2d,
                    in_offset=bass.IndirectOffsetOnAxis(ap=it[:, :1], axis=0),
                )
                nc.sync.dma_start(out=c_scr[ti * 128:(ti + 1) * 128], in_=cc)
            c_bs_i = tiny.tile([BC, S], i32)
            nc.sync.dma_start(out=c_bs_i, in_=c_scr.rearrange("(s b) -> b s", b=BC))
            ti32 = tiny.tile([BC, S], i32)
            nc.gpsimd.iota(ti32, pattern=[[1, S]], base=0, channel_multiplier=0)

            # -- dec-only DVE chain
            zeros = tiny.tile([BC, S], f32)
            nc.vector.memset(zeros, 0.0)
            sep_t = tiny.tile([BC, S], f32)
            nc.vector.tensor_scalar(sep_t, dec_bs, float(SEP), None, Alu.is_equal)
            oms = tiny.tile([BC, S], f32)
            nc.vector.tensor_scalar(oms, sep_t, -1.0, 1.0, Alu.mult, Alu.add)
            sepm = tiny.tile([BC, S], f32)
            nc.vector.tensor_copy(out=sepm, in_=sep_t)
            nc.vector.memset(sepm[:, 0:1], 1.0)
            segc = tiny.tile([BC, S], f32)
            nc.vector.tensor_tensor_scan(
                out=segc, data0=sep_t, data1=zeros, initial=0.0,
                op0=Alu.add, op1=Alu.add,
            )
            seg = tiny.tile([BC, S], f32)
            nc.vector.tensor_scalar(seg, segc, 5.0, None, Alu.min)
            # pattern [5,7,5,7,7,0] == 7 - 2*[seg==0] - 2*[seg==2] - 7*[seg==5]
            e0 = tiny.tile([BC, S], f32)
            nc.vector.tensor_scalar(e0, seg, 0.0, -2.0, Alu.is_equal, Alu.mult)
            e2 = tiny.tile([BC, S], f32)
            nc.vector.tensor_scalar(e2, seg, 2.0, -2.0, Alu.is_equal, Alu.mult)
            e5 = tiny.tile([BC, S], f32)
            nc.vector.tensor_scalar(e5, seg, 5.0, -7.0, Alu.is_equal, Alu.mult)
            pat = tiny.tile([BC, S], f32)
            nc.vector.tensor_tensor(out=pat, in0=e0, in1=e2, op=Alu.add)
            nc.vector.tensor_scalar(e5, e5, 7.0, None, Alu.add)
            nc.vector.tensor_tensor(out=pat, in0=pat, in1=e5, op=Alu.add)
            tf = tiny.tile([BC, S], f32)
            nc.vector.tensor_copy(out=tf, in_=ti32)
            t256 = tiny.tile([BC, S], f32)
            nc.vector.tensor_scalar(t256, tf, 256.0, None, Alu.mult)
            # F2 = 256t*sepm + sepm - 1 ; D2 = running max = 256*r
            F2 = tiny.tile([BC, S], f32)
            nc.vector.tensor_tensor(out=F2, in0=t256, in1=sepm, op=Alu.mult)
            nc.vector.scalar_tensor_tensor(
                out=F2, in0=F2, scalar=-1.0, in1=sepm,
                op0=Alu.add, op1=Alu.add,
            )
            D2 = tiny.tile([BC, S], f32)
            nc.vector.tensor_tensor_scan(
                out=D2, data0=F2, data1=zeros, initial=-1.0,
                op0=Alu.max, op1=Alu.add,
            )

            # -- gather-dependent DVE chain
            cf = tiny.tile([BC, S], f32)
            nc.vector.tensor_copy(out=cf, in_=c_bs_i)
            cm = tiny.tile([BC, S], f32)
            nc.vector.tensor_tensor(out=cm, in0=cf, in1=oms, op=Alu.mult)
            nc.vector.memset(cm[:, 0:1], 0.0)
            Ccum = tiny.tile([BC, S], f32)
            nc.vector.tensor_tensor_scan(
                out=Ccum, data0=cm, data1=zeros, initial=0.0,
                op0=Alu.add, op1=Alu.add,
            )
            # F = (256t + pat + C)*sepm + sepm - 1 ; D = running max
            F = tiny.tile([BC, S], f32)
            nc.vector.tensor_tensor(out=F, in0=pat, in1=Ccum, op=Alu.add)
            nc.vector.tensor_tensor(out=F, in0=F, in1=t256, op=Alu.add)
            nc.vector.tensor_tensor(out=F, in0=F, in1=sepm, op=Alu.mult)
            nc.vector.scalar_tensor_tensor(
                out=F, in0=F, scalar=-1.0, in1=sepm,
                op0=Alu.add, op1=Alu.add,
            )
            D = tiny.tile([BC, S], f32)
            nc.vector.tensor_tensor_scan(
                out=D, data0=F, data1=zeros, initial=-1.0,
                op0=Alu.max, op1=Alu.add,
            )
            # q4 = 4*max((D - D2) - C, 0); the w2s side carries the -2 shift
            q4 = tiny.tile([BC, S], f32)
            nc.vector.tensor_tensor(out=q4, in0=D, in1=D2, op=Alu.subtract)
            nc.vector.tensor_tensor(out=q4, in0=q4, in1=Ccum, op=Alu.subtract)
            nc.vector.tensor_scalar(q4, q4, 0.0, 4.0, Alu.max, Alu.mult)

            # [BC, S] -> [128, 2] threshold columns (p = s*BC + b; col = sb)
            q_scr = dram_pool.tile([ROWS], f32)
            nc.sync.dma_start(
                out=q_scr.rearrange("(s b) -> b s", b=BC), in_=q4
            )
            qc2 = tiny.tile([128, 2], f32)
            qc2_dma = nc.sync.dma_start(
                out=qc2, in_=q_scr.rearrange("(c p) -> p c", c=2)
            )
            qcols = [qc2[:, 0:1], qc2[:, 1:2]]

            # ---- persistent: broadcast (4*w2s - 2) across 128 partitions.
            # Valid matmul base partitions are only {0,32,64}: stage each
            # 2000-wide chunk as 2 rows (partitions 0/32) x 2 col-blocks,
            # loaded with a single DMA to keep PE wait fan-in small. bf16
            # operands keep PE at full rate (fp32 matmul is 4x slower).
            ones_tile = w_pool.tile([128, 128], bf16)
            nc.vector.memset(ones_tile, 1.0)
            neg2_col = w_pool.tile([128, 1], f32)
            nc.vector.memset(neg2_col, -2.0)
            w2s_bc = w_pool.tile([128, V], bf16)

            w2s_bf = w_pool.tile([128, 250], bf16)
            nc.gpsimd.dma_start(
                out=w2s_bf, in_=w2s.ap().rearrange("(p j) -> p j", j=250)
            )
            rhs_pos = [(0, 0), (0, 500), (32, 0), (32, 500)]
            for g in range(16):                       # 16 chunks of 2000
                stage = w_pool.tile(
                    [128, 1000], bf16, tag="stage", bufs=2
                )
                nc.scalar.dma_start(
                    out=stage[0:33:32, :].rearrange("p (c n) -> p c n", n=500),
                    in_=w2s_bf[8 * g:8 * g + 8, :],
                )
                # one PSUM bank per 500-wide matmul output
                pw = psum_pool.tile([128, 4, 512], f32, tag="pw")
                for q, (row, col) in enumerate(rhs_pos):
                    nc.tensor.matmul(
                        pw[:, q, 0:500],
                        lhsT=ones_tile[row:row + 1, :],
                        rhs=stage[row:row + 1, col:col + 500],
                        start=True,
                        stop=True,
                    )
                nc.scalar.activation(
                    out=w2s_bc[:, g * 2000:(g + 1) * 2000].rearrange(
                        "p (q n) -> p q n", n=500
                    ),
                    in_=pw[:, :, 0:500],
                    func=mybir.ActivationFunctionType.Identity,
                    bias=neg2_col[:, 0:1],
                    scale=4.0,
                )

            # ---- main streamed masked-fill
            # in-DMAs issue via gpsimd (SWDGE), out-DMAs via scalar (HWDGE),
            # spreading per-DMA issue overhead across otherwise-idle engines.
            # h is bf16: +/-inf is exact in bf16 and min() upconverts.
            import bass_rust as _br

            for k, (ci, sb) in enumerate(tile_order):
                c0, w = col0[ci], CHUNKS[ci]
                if k in lts:
                    lt = lts[k]
                else:
                    lt = lt_pool.tile([128, w], f32, tag=f"lt{w}",
                                      name=f"lt{k}",
                                      bufs=4 if w < 4000 else 6)
                    in_dma = nc.gpsimd.dma_start(
                        out=lt, in_=lgf[sb * 128:(sb + 1) * 128, c0:c0 + w],
                    )
                    if k == HOLD_K:
                        # keep the shared DMA engines free for the tiny
                        # q_scr/qc2 threshold bounce before flooding them
                        # with the remaining logits stream
                        _br.add_dep_helper(
                            in_dma.ins, qc2_dma.ins, sync=True,
                            reason="let threshold bounce through before stream",
                        )
                h = h_pool.tile([128, w], bf16, tag=f"h{w}", bufs=4)
                nc.vector.tensor_scalar(
                    h, w2s_bc[:, c0:c0 + w],
                    qcols[sb][:, 0:1], BIGNEG,
                    Alu.subtract, Alu.mult,
                )
                nc.vector.tensor_tensor(out=lt, in0=lt, in1=h, op=Alu.min)
                nc.sync.dma_start(
                    out=outf[sb * 128:(sb + 1) * 128, c0:c0 + w], in_=lt,
                )
    nc.compile()
    return nc


def _get_nc():
    if "nc" not in _CACHE:
        _CACHE["nc"] = _build_nc()
    return _CACHE["nc"]


def _shard_inputs(logits, decoder_input, word2syllables):
    logits = np.asarray(logits, dtype=np.float32)
    decoder_input = np.asarray(decoder_input, dtype=np.int32)
    word2syllables = np.ascontiguousarray(
        np.asarray(word2syllables, dtype=np.int32)
    )
    in_maps = []
    for c in range(NCORES):
        in_maps.append({
            "logits": np.ascontiguousarray(
                logits[:, c * BC:(c + 1) * BC, :]
            ).reshape(ROWS, V),
            "dec": np.ascontiguousarray(decoder_input[:, c * BC:(c + 1) * BC]),
            "w2s": word2syllables,
        })
    return in_maps


def _run(inputs, trace=False):
    from concourse.bass_utils import run_bass_kernel_spmd

    in_maps = _shard_inputs(
        inputs["logits"], inputs["decoder_input"], inputs["word2syllables"]
    )
    nc = _get_nc()
    res = run_bass_kernel_spmd(
        nc, in_maps, core_ids=list(range(NCORES)), trace=trace
    )
    outs = [r["out"].reshape(S, BC, V) for r in res.results]
    full = np.concatenate(outs, axis=1)
    return full, res


def kernel(logits, decoder_input, word2syllables, sample_n_to_check=1, **_kw):
    n = int(np.asarray(sample_n_to_check))
    assert n <= 1, f"kernel hardcodes sample_n_to_check==1, got {n}"
    full, _ = _run({
        "logits": logits,
        "decoder_input": decoder_input,
        "word2syllables": word2syllables,
    })
    return full.reshape(S, max(n, 1) * B, V)
